# revision 12
# baseline (speedup 1.0000x reference)
"""Trainium2 Bass kernel for nn_EventDenoisingMamba (4-layer Mamba, B=2, L=4096,
DM=256, DI=512, DS=16, DC=4, DR=16) on 8 NeuronCores.

Sharding: core c -> batch b=c//4, d_inner quarter q=c%4 (128 channels).
All arithmetic in fp32 (the reference overflows fp32 into NaN; the finite
values are chaotic so anything below fp32 precision diverges badly).

Per layer:
  - in_proj+conv (conv folded into weights) for the LOCAL d-slice (PE fp32)
  - x_proj partial over local d -> AllReduce[48, L] over the 4-core group
  - dt = stable softplus (relu(x)+ln(1+exp(-|x|))) on ACT
  - SSM cube: 16x { dA=exp (ACT), dB=u*B (DVE), tensor_tensor_scan (DVE),
    hc=h*C (DVE), sum_s via identity-matmul PSUM accumulation (PE) }
  - gate, AllGather y2 [128, L] over the group -> local full out_proj (PE)
"""
import numpy as np
import ml_dtypes

B, L, F = 2, 4096, 4
DM, DI, DS, DC, NL, DR = 256, 512, 16, 4, 4, 16
P = 128
LC = 1024
NCH = L // LC
SUB = 512
NSUB = LC // SUB
NCORES = 8

_COMPILED = {}


def _build_nc():
    import concourse.bass as bass
    import concourse.bacc as bacc
    import concourse.tile as tile
    from concourse import mybir

    f32 = mybir.dt.float32
    AF = mybir.ActivationFunctionType
    OP = mybir.AluOpType

    nc = bacc.Bacc(num_devices=NCORES)

    din = {}
    def inp(name, shape, dt=f32):
        din[name] = nc.dram_tensor(name, list(shape), dt, kind="ExternalInput")
        return din[name]

    inp("featT", (F, L))
    inp("embWT", (F, DM))
    inp("embb", (P, 2))
    inp("E", (P, NL * DC * 2 * P))       # local in_proj+conv lhsT, k-major
    inp("WzT", (P, NL * 2 * P))
    inp("WxT", (P, NL * 48))             # local x_proj lhsT (k = local d)
    inp("WdtT", (DR, NL * P))
    inp("WoT", (P, NL * 4 * DM))
    inp("headWT", (P, 2))
    inp("Acol", (P, NL * DS))
    inp("bc", (P, NL))
    inp("bdt", (P, NL))
    inp("Dsk", (P, NL))
    inp("ident", (P, P))
    inp("headb", (1, 1))

    out_t = nc.dram_tensor("out", [1, L], f32, kind="ExternalOutput")

    ar_in = nc.dram_tensor("ar_in", [48, L], f32)
    ar_out = nc.dram_tensor("ar_out", [48, L], f32)
    ag_in = nc.dram_tensor("ag_in", [P, L], f32)
    xh_d = nc.dram_tensor("xh_d", [P, L], f32)
    zg_d = nc.dram_tensor("zg_d", [P, L], f32)
    ag_out = nc.dram_tensor("ag_out", [4, P, L], f32)

    MY_GROUPS = [[0, 1, 2, 3], [4, 5, 6, 7]]

    with tile.TileContext(nc) as tc:
        with (
            tc.tile_pool(name="wts", bufs=1) as wts,
            tc.tile_pool(name="acts", bufs=1) as acts,
            tc.tile_pool(name="chunk", bufs=2) as chk,
            tc.tile_pool(name="sloop", bufs=2) as sl,
            tc.tile_pool(name="ps_proj", bufs=3, space="PSUM") as psp,
            tc.tile_pool(name="ps_y", bufs=4, space="PSUM") as psy,
        ):
            # ---------------- static weights -----------------------------
            embWT = wts.tile([F, DM], f32)
            nc.sync.dma_start(embWT[:], din["embWT"][:])
            embb = wts.tile([P, 2], f32)
            nc.sync.dma_start(embb[:], din["embb"][:])
            ident = wts.tile([P, P], f32)
            nc.sync.dma_start(ident[:], din["ident"][:])
            headWT = wts.tile([P, 2], f32)
            nc.sync.dma_start(headWT[:], din["headWT"][:])
            headb = wts.tile([1, 1], f32)
            nc.sync.dma_start(headb[:], din["headb"][:])
            E_sb = wts.tile([P, NL * DC * 2 * P], f32)
            nc.sync.dma_start(E_sb[:], din["E"][:])
            WzT = wts.tile([P, NL * 2 * P], f32)
            nc.sync.dma_start(WzT[:], din["WzT"][:])
            WxT = wts.tile([P, NL * 48], f32)
            nc.sync.dma_start(WxT[:], din["WxT"][:])
            WdtT = wts.tile([DR, NL * P], f32)
            nc.sync.dma_start(WdtT[:], din["WdtT"][:])
            WoT = wts.tile([P, NL * 4 * DM], f32)
            nc.sync.dma_start(WoT[:], din["WoT"][:])
            Acol = wts.tile([P, NL * DS], f32)
            nc.sync.dma_start(Acol[:], din["Acol"][:])
            bc = wts.tile([P, NL], f32)
            nc.sync.dma_start(bc[:], din["bc"][:])
            bdt = wts.tile([P, NL], f32)
            nc.sync.dma_start(bdt[:], din["bdt"][:])
            Dsk = wts.tile([P, NL], f32)
            nc.sync.dma_start(Dsk[:], din["Dsk"][:])

            # ---------------- persistent activations ---------------------
            LPAD = L + 4
            x0 = acts.tile([P, LPAD], f32)
            x1 = acts.tile([P, LPAD], f32)
            xs = [x0, x1]
            nc.vector.memset(x0[:, 0:4], 0.0)
            nc.vector.memset(x1[:, 0:4], 0.0)
            h_tail = acts.tile([P, DS], f32)

            # ---------------- embed --------------------------------------
            for c8 in range(L // SUB):
                featc = chk.tile([F, SUB], f32, tag="prj")
                nc.sync.dma_start(featc[:],
                                  din["featT"][:, c8 * SUB:(c8 + 1) * SUB])
                for m in range(2):
                    pse = psp.tile([P, SUB], f32, tag="ps")
                    nc.tensor.matmul(pse[:], embWT[:, m * P:(m + 1) * P],
                                     featc[:], start=True, stop=True)
                    nc.scalar.activation(
                        xs[m][:, 3 + c8 * SUB:3 + (c8 + 1) * SUB], pse[:],
                        AF.Identity, bias=embb[:, m:m + 1])

            # ---------------- layers -------------------------------------
            for l in range(NL):
                # ===== phase A: xh (local), z, x_proj partial =========
                for ch in range(NCH):
                    t0 = ch * LC
                    xh_a = chk.tile([P, LC], f32, tag="xh_a", bufs=1)
                    zg_a = chk.tile([P, LC], f32, tag="zg_a", bufs=1)
                    for sub in range(NSUB):
                        s0 = t0 + sub * SUB
                        psx = psp.tile([P, SUB], f32, tag="ps")
                        n_mm = 0
                        for tap in range(DC):
                            for kt in range(2):
                                eidx = (l * DC + tap) * 2 + kt
                                nc.tensor.matmul(
                                    psx[:],
                                    E_sb[:, eidx * P:(eidx + 1) * P],
                                    xs[kt][:, s0 + tap: s0 + tap + SUB],
                                    start=(n_mm == 0), stop=(n_mm == 7))
                                n_mm += 1
                        nc.scalar.activation(
                            xh_a[:, sub * SUB:(sub + 1) * SUB], psx[:],
                            AF.Silu, bias=bc[:, l:l + 1])
                        psz = psp.tile([P, SUB], f32, tag="ps")
                        for kt in range(2):
                            nc.tensor.matmul(
                                psz[:],
                                WzT[:, (l * 2 + kt) * P:(l * 2 + kt + 1) * P],
                                xs[kt][:, s0 + 3: s0 + 3 + SUB],
                                start=(kt == 0), stop=(kt == 1))
                        nc.scalar.activation(
                            zg_a[:, sub * SUB:(sub + 1) * SUB], psz[:],
                            AF.Silu)
                        psj = psp.tile([48, SUB], f32, tag="ps")
                        nc.tensor.matmul(
                            psj[:], WxT[:, l * 48:(l + 1) * 48],
                            xh_a[:, sub * SUB:(sub + 1) * SUB],
                            start=True, stop=True)
                        prj = chk.tile([48, SUB], f32, tag="prj")
                        nc.scalar.activation(prj[:], psj[:], AF.Copy)
                        nc.sync.dma_start(ar_in[:, s0:s0 + SUB], prj[:])
                    nc.sync.dma_start(xh_d[:, t0:t0 + LC], xh_a[:])
                    nc.sync.dma_start(zg_d[:, t0:t0 + LC], zg_a[:])
                # ===== phase B: AllReduce x_proj partials =============
                nc.gpsimd.collective_compute(
                    "AllReduce", OP.add, replica_groups=MY_GROUPS,
                    ins=[ar_in[:]], outs=[ar_out[:]])
                # ===== phase C: dt, cube, gate ========================
                for ch in range(NCH):
                    t0 = ch * LC
                    proj16 = chk.tile([DR, LC], f32, tag="proj16")
                    nc.sync.dma_start(proj16[:], ar_out[0:DR, t0:t0 + LC])
                    xh_l = chk.tile([P, LC], f32, tag="xh_l")
                    nc.sync.dma_start(xh_l[:], xh_d[:, t0:t0 + LC])
                    zg_c = chk.tile([P, LC], f32, tag="zg_c")
                    nc.sync.dma_start(zg_c[:], zg_d[:, t0:t0 + LC])
                    dt_f = chk.tile([P, LC], f32, tag="dt_f")
                    for sub in range(NSUB):
                        s0 = t0 + sub * SUB
                        psd = psp.tile([P, SUB], f32, tag="ps")
                        nc.tensor.matmul(
                            psd[:], WdtT[:, l * P:(l + 1) * P],
                            proj16[:, sub * SUB:(sub + 1) * SUB],
                            start=True, stop=True)
                        ab = chk.tile([P, SUB], f32, tag="ab")
                        rl = chk.tile([P, SUB], f32, tag="rl")
                        nc.scalar.activation(ab[:], psd[:], AF.Abs,
                                             bias=bdt[:, l:l + 1])
                        nc.scalar.activation(rl[:], psd[:], AF.Relu,
                                             bias=bdt[:, l:l + 1])
                        nc.scalar.activation(ab[:], ab[:], AF.Exp, scale=-1.0)
                        nc.scalar.activation(ab[:], ab[:], AF.Ln, bias=1.0)
                        nc.vector.tensor_tensor(
                            dt_f[:, sub * SUB:(sub + 1) * SUB], ab[:], rl[:],
                            OP.add)
                    u_f = chk.tile([P, LC], f32, tag="u_f")
                    nc.vector.tensor_tensor(u_f[:], dt_f[:], xh_l[:],
                                            OP.mult)
                    psy_t = [psy.tile([P, SUB], f32, tag="psy",
                                      name=f"psy{i}") for i in range(NSUB)]
                    for s in range(DS):
                        brep = sl.tile([P, LC], f32, tag="sl_a")
                        crep = sl.tile([P, LC], f32, tag="sl_c")
                        brow = ar_out[DR + s:DR + s + 1, t0:t0 + LC]
                        crow = ar_out[DR + DS + s:DR + DS + s + 1, t0:t0 + LC]
                        nc.sync.dma_start(brep[:], bass.AP(
                            tensor=brow.tensor, offset=brow.offset,
                            ap=[[0, P]] + list(brow.ap)[1:]))
                        nc.sync.dma_start(crep[:], bass.AP(
                            tensor=crow.tensor, offset=crow.offset,
                            ap=[[0, P]] + list(crow.ap)[1:]))
                        dA = sl.tile([P, LC], f32, tag="sl_d")
                        nc.scalar.activation(
                            dA[:], dt_f[:], AF.Exp,
                            scale=Acol[:, l * DS + s:l * DS + s + 1])
                        dB = sl.tile([P, LC], f32, tag="sl_b")
                        nc.vector.tensor_tensor(dB[:], u_f[:], brep[:],
                                                OP.mult)
                        h = sl.tile([P, LC], f32, tag="sl_a")
                        nc.vector.tensor_tensor_scan(
                            h[:], dA[:], dB[:],
                            0.0 if ch == 0 else h_tail[:, s:s + 1],
                            OP.mult, OP.add)
                        if ch < NCH - 1:
                            nc.vector.tensor_copy(h_tail[:, s:s + 1],
                                                  h[:, LC - 1:LC])
                        hc = sl.tile([P, LC], f32, tag="sl_b")
                        nc.vector.tensor_tensor(hc[:], h[:], crep[:], OP.mult)
                        for sub in range(NSUB):
                            nc.tensor.matmul(
                                psy_t[sub][:], ident[:],
                                hc[:, sub * SUB:(sub + 1) * SUB],
                                start=(s == 0), stop=(s == DS - 1))
                    y2 = chk.tile([P, LC], f32, tag="y2", bufs=1)
                    for sub in range(NSUB):
                        t1 = chk.tile([P, SUB], f32, tag="t1")
                        nc.vector.scalar_tensor_tensor(
                            t1[:], xh_l[:, sub * SUB:(sub + 1) * SUB],
                            Dsk[:, l:l + 1], psy_t[sub][:], OP.mult, OP.add)
                        nc.vector.tensor_tensor(
                            y2[:, sub * SUB:(sub + 1) * SUB], t1[:],
                            zg_c[:, sub * SUB:(sub + 1) * SUB], OP.mult)
                    nc.gpsimd.dma_start(ag_in[:, t0:t0 + LC], y2[:])
                # ===== phase D: AllGather y2 ==========================
                nc.gpsimd.collective_compute(
                    "AllGather", OP.bypass, replica_groups=MY_GROUPS,
                    ins=[ag_in[:]], outs=[ag_out[:]])
                # ===== phase E: out_proj ==============================
                for ch in range(NCH):
                    t0 = ch * LC
                    for sub in range(NSUB):
                        s0 = t0 + sub * SUB
                        yg = [chk.tile([P, SUB], f32, tag=f"yg{k}",
                                       name=f"yg{k}") for k in range(4)]
                        for k in range(4):
                            nc.sync.dma_start(yg[k][:],
                                              ag_out[k, :, s0:s0 + SUB])
                        for m in range(2):
                            pso = psp.tile([P, SUB], f32, tag="ps")
                            for kt in range(4):
                                nc.tensor.matmul(
                                    pso[:],
                                    WoT[:, (l * 4 + kt) * DM + m * P:
                                        (l * 4 + kt) * DM + (m + 1) * P],
                                    yg[kt][:], start=(kt == 0),
                                    stop=(kt == 3))
                            nc.scalar.activation(
                                xs[m][:, 3 + s0:3 + s0 + SUB], pso[:],
                                AF.Copy)
            # ---------------- head ---------------------------------------
            for c8 in range(L // SUB):
                s0 = c8 * SUB
                psh = psp.tile([1, SUB], f32, tag="ps")
                for m in range(2):
                    nc.tensor.matmul(psh[:], headWT[:, m:m + 1],
                                     xs[m][:, 3 + s0:3 + s0 + SUB],
                                     start=(m == 0), stop=(m == 1))
                outc = chk.tile([1, SUB], f32, tag="outc")
                nc.scalar.activation(outc[:], psh[:],
                                     AF.Identity, bias=headb[:])
                nc.gpsimd.dma_start(out_t[:, s0:s0 + SUB], outc[:])

    nc.compile()
    return nc


def _prep_inputs(inputs):
    f32 = np.float32
    A_full = -np.exp(inputs["A_log"]).astype(f32)
    Win = inputs["in_proj_W"].astype(f32)
    Wc = inputs["conv_W"].astype(f32)
    Wx = inputs["x_proj_W"].astype(f32)
    Wdt = inputs["dt_proj_W"].astype(f32)
    Wo = inputs["out_proj_W"].astype(f32)
    ident = np.eye(P, dtype=f32)

    maps = []
    for c in range(NCORES):
        b, q = c // 4, c % 4
        dsl = slice(q * P, (q + 1) * P)
        E = np.empty((NL, DC, 2, P, P), f32)
        for l in range(NL):
            for tap in range(DC):
                M = Win[l, :DI, :][dsl] * Wc[l, dsl, tap:tap + 1]  # [128,256]
                MT = M.T                                           # [256,128]
                for kt in range(2):
                    E[l, tap, kt] = MT[kt * P:(kt + 1) * P]
        maps.append({
            "featT": np.ascontiguousarray(inputs["features"][b].T).astype(f32),
            "embWT": np.ascontiguousarray(inputs["embed_W"].T).astype(f32),
            "embb": np.ascontiguousarray(
                inputs["embed_b"].reshape(2, P).T).astype(f32),
            "E": np.ascontiguousarray(
                E.transpose(3, 0, 1, 2, 4).reshape(P, -1)).astype(f32),
            "WzT": np.ascontiguousarray(np.stack(
                [Win[:, DI + q * P:DI + (q + 1) * P,
                     kt * P:(kt + 1) * P].transpose(0, 2, 1)
                 for kt in range(2)], axis=1)
                .transpose(2, 0, 1, 3).reshape(P, -1)).astype(f32),
            "WxT": np.ascontiguousarray(
                Wx[:, :, dsl].transpose(2, 0, 1).reshape(P, -1)).astype(f32),
            "WdtT": np.ascontiguousarray(
                Wdt[:, dsl, :].transpose(2, 0, 1).reshape(DR, -1)).astype(f32),
            "WoT": np.ascontiguousarray(np.stack(
                [Wo[:, :, k * P:(k + 1) * P].transpose(0, 2, 1)
                 for k in range(4)], axis=1)
                .transpose(2, 0, 1, 3).reshape(P, -1)).astype(f32),
            "headWT": np.ascontiguousarray(
                inputs["head_W"].reshape(2, P).T).astype(f32),
            "Acol": np.ascontiguousarray(
                A_full[:, dsl, :].transpose(1, 0, 2).reshape(P, -1)
            ).astype(f32),
            "bc": np.ascontiguousarray(inputs["conv_b"][:, dsl].T).astype(f32),
            "bdt": np.ascontiguousarray(
                inputs["dt_proj_b"][:, dsl].T).astype(f32),
            "Dsk": np.ascontiguousarray(
                inputs["D_skip"][:, dsl].T).astype(f32),
            "ident": ident,
            "headb": np.array([[float(inputs["head_b"][0])]], f32),
        })
    return maps


def kernel(**inputs):
    from concourse.bass_utils import run_bass_kernel_spmd
    if "nc" not in _COMPILED:
        _COMPILED["nc"] = _build_nc()
    nc = _COMPILED["nc"]
    inputs = {k: np.asarray(v) for k, v in inputs.items()}
    in_maps = _prep_inputs(inputs)
    res = run_bass_kernel_spmd(nc, in_maps, core_ids=list(range(NCORES)))
    out = np.zeros((B, L, 1), np.float32)
    out[0, :, 0] = res.results[0]["out"][0]
    out[1, :, 0] = res.results[4]["out"][0]
    return out
